# revision 21
# baseline (speedup 1.0000x reference)
"""Trainium2 Bass kernel for the all-pairs spring-energy sum (EnergyLossVectorized).

Contract: kernel(**inputs) takes FULL unsharded inputs (p [32768,2] f32,
edge_attr [E,2] f32, src/dst [E] i32 with E = 64*512*511), returns the FULL
scalar output, distributing across 8 NeuronCores internally.

Strategy: src/dst produced by the reference's setup_inputs() are the
deterministic all-directed-pairs (i != j) indices per graph, in i-major
order.  We verify that structure on the host (falling back to a straight
numpy evaluation if it ever doesn't hold) and then compute the energy with
a gather-free formulation:

  For each graph g (512 nodes), the 512x512 grid D2[i,j] = |p_i - p_j|^2 is
  computed on the tensor engine as a K=8 matmul  D2 = PL^T @ PR with
     PL features: [ x,  y,  rhi, rmid, rlo, 1, 1, 1 ]
     PR features: [-2x, -2y, 1,   1,   1,   rhi, rmid, rlo ]
  where r = x^2 + y^2 of the bf16-rounded coords is carried as three bf16
  limbs, so the PSUM result equals |p_i - p_j|^2 to ~fp32 accuracy (no
  cancellation blowup), guaranteeing D2 >= -1e-5 and sqrt(D2+EPS) NaN-free.

  edge_attr (l, k) is re-laid-out on the host into per-graph [512,512] bf16
  grids with k=0 on the diagonal, interleaved [p, {l,k}, t, j] so each
  graph is a single contiguous 1 MB DMA.  Per half-graph tile [128 x 1024]:
     s  = sqrt(D2 + EPS)            (scalar engine, PSUM -> SBUF bf16)
     u  = s - l                     (DVE / GPSIMD)
     e  = (u ^ 2) * k  + row-sum    (DVE scalar_tensor_tensor, accum_out)
  Per-row partials accumulate in parts[128, 16]; the final reduction is one
  tensor_reduce + a [1x1] ones-matmul + 0.5 scale on device; the host sums
  the 8 per-core scalars.

Memory traffic per core: 8 graphs * 1 MB = 8.4 MB bf16 -> ~24 us roofline.
"""

import os
import sys

import numpy as np

for _p in ("/opt/trn_rl_repo", "/root/.axon_site/_ro/trn_rl_repo"):
    if os.path.isdir(_p) and _p not in sys.path:
        sys.path.insert(0, _p)

import ml_dtypes

bf16 = ml_dtypes.bfloat16

NUM_GRAPHS = 64
N = 512                      # nodes per graph
NCORES = 8
GPC = NUM_GRAPHS // NCORES   # graphs per core = 8
PB = 128                     # partition block (i-tile)
EPS = 1e-5                   # sqrt clamp; D2 >= -1e-5 guaranteed by 3-limb r

# elementwise engine split: every SUB_GPS-th half-tile's subtract runs on
# GPSIMD to offload the vector engine
SUB_GPS = 3
# e = (u^2)*k implementation: "pow" = single fused scalar_tensor_tensor with
# pow ALU op; "mul" = v=u*k on DVE/GPSIMD then fused (v*u)+reduce
SQ_MODE = "mul"


def _build_nc(gpc=GPC, n=N, pb=PB, debug=False):
    """Build + compile the per-core Bass program (SPMD, same on all cores)."""
    import concourse.bass as bass
    import concourse.tile as tile
    from concourse import bacc, mybir

    tb = n // pb             # i-tiles per graph (4)
    th = tb // 2             # halves per graph (2), each [pb, 2, n]
    fdt = mybir.dt.float32
    bdt = mybir.dt.bfloat16
    AF = mybir.ActivationFunctionType
    AL = mybir.AluOpType

    nc = bacc.Bacc("TRN2", target_bir_lowering=False, debug=debug,
                   num_devices=NCORES)

    # lk: [graph, partition, {l,k}, t, j] so one graph = 1 contiguous DMA
    lk_d = nc.dram_tensor("lk", [gpc, pb, 2, tb, n], bdt, kind="ExternalInput")
    pl_d = nc.dram_tensor("plin", [64, 4 * n], bdt, kind="ExternalInput")
    pr_d = nc.dram_tensor("prin", [64, 4 * n], bdt, kind="ExternalInput")
    out_d = nc.dram_tensor("out", [1, 1], fdt, kind="ExternalOutput")

    lk = lk_d.ap()

    with tile.TileContext(nc) as tc:
        with (
            tc.tile_pool(name="const", bufs=1) as const,
            tc.tile_pool(name="lkp", bufs=3) as lkp,
            tc.tile_pool(name="work", bufs=3) as work,
            tc.tile_pool(name="psum", bufs=3, space="PSUM") as psum,
            tc.tile_pool(name="accp", bufs=1, space="PSUM") as accp,
        ):
            # host-precomputed matmul operands (see _build_plt_prt)
            plt = const.tile([64, 4 * n], bdt)
            prt = const.tile([64, 4 * n], bdt)
            nc.sync.dma_start(plt[:], pl_d.ap())
            nc.sync.dma_start(prt[:], pr_d.ap())

            ones_col = const.tile([pb, 1], fdt)
            nc.vector.memset(ones_col[:], 1.0)
            eps_col = const.tile([pb, 1], fdt)
            nc.vector.memset(eps_col[:], EPS)

            parts = const.tile([pb, gpc * th], fdt)

            idx = 0
            for g in range(gpc):
                g_, gg = divmod(g, 4)
                lkt = lkp.tile([pb, 2, tb, n], bdt)
                nc.sync.dma_start(lkt[:], lk[g])
                for h in range(th):
                    ps = psum.tile([pb, 2, n], fdt)
                    for tt in range(2):
                        t = 2 * h + tt
                        nc.tensor.matmul(
                            ps[:, tt, :],
                            plt[32 * g_:32 * g_ + 8,
                                gg * n + t * pb: gg * n + (t + 1) * pb],
                            prt[32 * g_:32 * g_ + 8, gg * n:(gg + 1) * n],
                            start=True, stop=True,
                        )
                    s = work.tile([pb, 2, n], bdt, tag="s")
                    nc.scalar.activation(s[:], ps[:], AF.Sqrt, bias=eps_col[:])
                    u = work.tile([pb, 2, n], bdt, tag="u")
                    lsl = lkt[:, 0, 2 * h:2 * h + 2, :]
                    ksl = lkt[:, 1, 2 * h:2 * h + 2, :]
                    if idx % SUB_GPS == SUB_GPS - 1:
                        nc.gpsimd.tensor_sub(u[:], s[:], lsl)
                    else:
                        nc.vector.tensor_sub(u[:], s[:], lsl)
                    e = work.tile([pb, 2, n], bdt, tag="e")
                    if SQ_MODE == "pow":
                        # e = (u^2) * k ; parts[:, idx] = row-sum(e)
                        nc.vector.scalar_tensor_tensor(
                            e[:], u[:], 2.0, ksl,
                            op0=AL.pow, op1=AL.mult,
                            accum_out=parts[:, idx:idx + 1],
                        )
                    else:
                        v = work.tile([pb, 2, n], bdt, tag="v")
                        if idx % 3 == 1:
                            nc.gpsimd.tensor_mul(v[:], u[:], ksl)
                        else:
                            nc.vector.tensor_mul(v[:], u[:], ksl)
                        nc.vector.scalar_tensor_tensor(
                            e[:], v[:], 1.0, u[:],
                            op0=AL.mult, op1=AL.mult,
                            accum_out=parts[:, idx:idx + 1],
                        )
                    idx += 1

            # ---- final reduction to a scalar ----
            pr1 = const.tile([pb, 1], fdt)
            nc.vector.tensor_reduce(
                pr1[:], parts[:], axis=mybir.AxisListType.X, op=AL.add)
            acc11 = accp.tile([1, 1], fdt)
            nc.tensor.matmul(acc11[:], ones_col[:], pr1[:],
                             start=True, stop=True)
            tot = const.tile([1, 1], fdt)
            nc.vector.tensor_scalar_mul(tot[:], acc11[:], 0.5)
            nc.sync.dma_start(out_d.ap(), tot[:])

    nc.compile()
    return nc


_NC_CACHE = {}


def _get_nc(gpc=GPC, n=N, pb=PB):
    key = (gpc, n, pb)
    if key not in _NC_CACHE:
        _NC_CACHE[key] = _build_nc(gpc, n, pb)
    return _NC_CACHE[key]


def _expected_pairs(num_graphs, n):
    i = np.repeat(np.arange(n, dtype=np.int64), n)
    j = np.tile(np.arange(n, dtype=np.int64), n)
    keep = i != j
    si, sj = i[keep], j[keep]
    off = (np.arange(num_graphs, dtype=np.int64) * n)[:, None]
    src = (off + si[None, :]).reshape(-1)
    dst = (off + sj[None, :]).reshape(-1)
    return src.astype(np.int32), dst.astype(np.int32)


def _structure_ok(src, dst):
    if src.shape != (NUM_GRAPHS * N * (N - 1),):
        return False
    esrc, edst = _expected_pairs(NUM_GRAPHS, N)
    return np.array_equal(src, esrc) and np.array_equal(dst, edst)


def _fallback_numpy(p, edge_attr, src, dst):
    start = p[src].astype(np.float64)
    end = p[dst].astype(np.float64)
    t12 = ((start - end) ** 2).sum(axis=1)
    l = edge_attr[:, 0].astype(np.float64)
    k = edge_attr[:, 1].astype(np.float64)
    energy = k / 2.0 * (t12 + l * l - 2.0 * l * np.sqrt(t12))
    return np.float32(energy.sum())


def _build_plt_prt(p_core, gpc=GPC, n=N):
    """p_core [gpc*n, 2] f32 -> (plt, prt) [64, 4n] bf16 matmul operands."""
    xb = p_core.reshape(gpc, n, 2).astype(bf16)          # bf16-rounded coords
    xf = xb[..., 0].astype(np.float32)
    yf = xb[..., 1].astype(np.float32)
    r = xf * xf + yf * yf
    rhi = r.astype(bf16)
    r1 = r - rhi.astype(np.float32)
    rmid = r1.astype(bf16)
    r2 = r1 - rmid.astype(np.float32)
    rlo = r2.astype(bf16)
    plt = np.ones((64, 4 * n), dtype=bf16)
    prt = np.ones((64, 4 * n), dtype=bf16)
    feats_l = [xb[..., 0], xb[..., 1], rhi, rmid, rlo]
    feats_r = [(xb[..., 0] * bf16(-2.0)), (xb[..., 1] * bf16(-2.0)),
               None, None, None, rhi, rmid, rlo]
    for g in range(gpc):
        g_, gg = divmod(g, 4)
        cols = slice(gg * n, (gg + 1) * n)
        for f, arr in enumerate(feats_l):
            plt[32 * g_ + f, cols] = arr[g]
        for f, arr in enumerate(feats_r):
            if arr is not None:
                prt[32 * g_ + f, cols] = arr[g]
    return plt, prt


def _build_grids(edge_attr):
    """edge_attr [E,2] f32 -> lk bf16 array [NCORES, GPC, PB, 2, TB, N]."""
    tb = N // PB
    ea = edge_attr.astype(bf16).reshape(NUM_GRAPHS, N * (N - 1), 2)
    offdiag = (~np.eye(N, dtype=bool)).reshape(-1)
    grid = np.zeros((2, NUM_GRAPHS, N * N), dtype=bf16)
    grid[0][:, offdiag] = ea[:, :, 0]
    grid[1][:, offdiag] = ea[:, :, 1]
    # [2, graphs, t, p, j] -> [cores, gpc, p, 2, t, j]
    g5 = grid.reshape(2, NUM_GRAPHS, tb, PB, N)
    lk = np.ascontiguousarray(g5.transpose(1, 3, 0, 2, 4))  # [G, PB, 2, tb, N]
    return lk.reshape(NCORES, GPC, PB, 2, tb, N)


def kernel(p, edge_attr, src, dst):
    p = np.ascontiguousarray(np.asarray(p, dtype=np.float32))
    edge_attr = np.ascontiguousarray(np.asarray(edge_attr, dtype=np.float32))
    src = np.asarray(src, dtype=np.int32)
    dst = np.asarray(dst, dtype=np.int32)

    if not _structure_ok(src, dst):
        return _fallback_numpy(p, edge_attr, src, dst)

    from concourse.bass_utils import run_bass_kernel_spmd

    lk = _build_grids(edge_attr)
    pcs = p.reshape(NCORES, GPC * N, 2)

    nc = _get_nc()
    in_maps = []
    for c in range(NCORES):
        plt, prt = _build_plt_prt(pcs[c])
        in_maps.append({"lk": lk[c], "plin": plt, "prin": prt})
    res = run_bass_kernel_spmd(nc, in_maps, list(range(NCORES)))
    total = sum(float(res.results[c]["out"][0, 0]) for c in range(NCORES))
    return np.float32(total)


if __name__ == "__main__":
    nc = _get_nc()
    print("compiled ok")


# revision 27
# speedup vs baseline: 1.0237x; 1.0237x over previous
"""Trainium2 Bass kernel for the all-pairs spring-energy sum (EnergyLossVectorized).

Contract: kernel(**inputs) takes FULL unsharded inputs (p [32768,2] f32,
edge_attr [E,2] f32, src/dst [E] i32 with E = 64*512*511), returns the FULL
scalar output, distributing across 8 NeuronCores internally.

Strategy: src/dst produced by the reference's setup_inputs() are the
deterministic all-directed-pairs (i != j) indices per graph, in i-major
order.  We verify that structure on the host (falling back to a straight
numpy evaluation if it ever doesn't hold) and then compute the energy with
a gather-free formulation:

  For each graph g (512 nodes), the 512x512 grid D2[i,j] = |p_i - p_j|^2 is
  computed on the tensor engine as a K=8 matmul  D2 = PL^T @ PR with
     PL features: [ x,  y,  rhi, rmid, rlo, 1, 1, 1 ]
     PR features: [-2x, -2y, 1,   1,   1,   rhi, rmid, rlo ]
  where r = x^2 + y^2 of the bf16-rounded coords is carried as three bf16
  limbs, so the PSUM result equals |p_i - p_j|^2 to ~fp32 accuracy (no
  cancellation blowup), guaranteeing D2 >= -1e-5 and sqrt(D2+EPS) NaN-free.

  edge_attr (l, k) is re-laid-out on the host into per-graph [512,512] bf16
  grids with k=0 on the diagonal, interleaved [p, {l,k}, t, j] so each
  graph is a single contiguous 1 MB DMA.  Per half-graph tile [128 x 1024]:
     s  = sqrt(D2 + EPS)            (scalar engine, PSUM -> SBUF bf16)
     u  = s - l                     (DVE / GPSIMD)
     e  = (u ^ 2) * k  + row-sum    (DVE scalar_tensor_tensor, accum_out)
  Per-row partials accumulate in parts[128, 16]; the final reduction is one
  tensor_reduce + a [1x1] ones-matmul + 0.5 scale on device; the host sums
  the 8 per-core scalars.

Memory traffic per core: 8 graphs * 1 MB = 8.4 MB bf16 -> ~24 us roofline.
"""

import os
import sys

import numpy as np

for _p in ("/opt/trn_rl_repo", "/root/.axon_site/_ro/trn_rl_repo"):
    if os.path.isdir(_p) and _p not in sys.path:
        sys.path.insert(0, _p)

import ml_dtypes

bf16 = ml_dtypes.bfloat16

NUM_GRAPHS = 64
N = 512                      # nodes per graph
NCORES = 8
GPC = NUM_GRAPHS // NCORES   # graphs per core = 8
PB = 128                     # partition block (i-tile)
EPS = 1e-5                   # sqrt clamp; D2 >= -1e-5 guaranteed by 3-limb r

# per-half-tile engine assignment (16 half-tiles), tuned from profiles:
# sub u=s-l: D=vector, G=gpsimd ; v=u*k: D=vector, G=gpsimd, A=scalar(Square)
SUB_PAT = "DDGDDGDDGDDGDDGD"
SQ_PAT = "ADGADGADGADGADGA"


def _build_nc(gpc=GPC, n=N, pb=PB, debug=False):
    """Build + compile the per-core Bass program (SPMD, same on all cores)."""
    import concourse.bass as bass
    import concourse.tile as tile
    from concourse import bacc, mybir

    tb = n // pb             # i-tiles per graph (4)
    th = tb // 2             # halves per graph (2), each [pb, 2, n]
    fdt = mybir.dt.float32
    bdt = mybir.dt.bfloat16
    AF = mybir.ActivationFunctionType
    AL = mybir.AluOpType

    nc = bacc.Bacc("TRN2", target_bir_lowering=False, debug=debug,
                   num_devices=NCORES)

    # lk: [graph, partition, {l,k}, t*j] so one graph = 1 contiguous DMA
    lk_d = nc.dram_tensor("lk", [gpc, pb, 2, tb * n], bdt,
                          kind="ExternalInput")
    pl_d = nc.dram_tensor("plin", [64, 4 * n], bdt, kind="ExternalInput")
    pr_d = nc.dram_tensor("prin", [64, 4 * n], bdt, kind="ExternalInput")
    out_d = nc.dram_tensor("out", [1, 1], fdt, kind="ExternalOutput")

    lk = lk_d.ap()

    with tile.TileContext(nc) as tc:
        with (
            tc.tile_pool(name="const", bufs=1) as const,
            tc.tile_pool(name="lkp", bufs=3) as lkp,
            tc.tile_pool(name="work", bufs=3) as work,
            tc.tile_pool(name="psum", bufs=3, space="PSUM") as psum,
            tc.tile_pool(name="accp", bufs=1, space="PSUM") as accp,
        ):
            # host-precomputed matmul operands (see _build_plt_prt)
            plt = const.tile([64, 4 * n], bdt)
            prt = const.tile([64, 4 * n], bdt)
            nc.sync.dma_start(plt[:], pl_d.ap())
            nc.sync.dma_start(prt[:], pr_d.ap())

            ones_col = const.tile([pb, 1], fdt)
            nc.vector.memset(ones_col[:], 1.0)
            eps_col = const.tile([pb, 1], fdt)
            nc.vector.memset(eps_col[:], EPS)
            zero_col = const.tile([pb, 1], fdt)
            nc.vector.memset(zero_col[:], 0.0)

            parts = const.tile([pb, gpc * th], fdt)

            nh = 2 * n           # half-graph free width (flat 2D, step-1)
            idx = 0
            for g in range(gpc):
                g_, gg = divmod(g, 4)
                lkt = lkp.tile([pb, 2, tb * n], bdt)
                nc.sync.dma_start(lkt[:], lk[g])
                for h in range(th):
                    ps = psum.tile([pb, nh], fdt)
                    for tt in range(2):
                        t = 2 * h + tt
                        nc.tensor.matmul(
                            ps[:, tt * n:(tt + 1) * n],
                            plt[32 * g_:32 * g_ + 8,
                                gg * n + t * pb: gg * n + (t + 1) * pb],
                            prt[32 * g_:32 * g_ + 8, gg * n:(gg + 1) * n],
                            start=True, stop=True,
                        )
                    s = work.tile([pb, nh], bdt, tag="s")
                    nc.scalar.activation(s[:], ps[:], AF.Sqrt, bias=eps_col[:])
                    u = work.tile([pb, nh], bdt, tag="u")
                    lsl = lkt[:, 0, h * nh:(h + 1) * nh]
                    ksl = lkt[:, 1, h * nh:(h + 1) * nh]
                    if SUB_PAT[idx] == "G":
                        nc.gpsimd.tensor_sub(u[:], s[:], lsl)
                    else:
                        nc.vector.tensor_sub(u[:], s[:], lsl)
                    # e = k*u^2 with fused row-sum into parts[:, idx]
                    v = work.tile([pb, nh], bdt, tag="v")
                    e = work.tile([pb, nh], bdt, tag="e")
                    sq = SQ_PAT[idx]
                    if sq == "A":
                        nc.scalar.activation(v[:], u[:], AF.Square,
                                             bias=zero_col[:])
                        a0, a1 = v[:], ksl
                    else:
                        if sq == "G":
                            nc.gpsimd.tensor_mul(v[:], u[:], ksl)
                        else:
                            nc.vector.tensor_mul(v[:], u[:], ksl)
                        a0, a1 = v[:], u[:]
                    nc.vector.scalar_tensor_tensor(
                        e[:], a0, 1.0, a1,
                        op0=AL.mult, op1=AL.mult,
                        accum_out=parts[:, idx:idx + 1],
                    )
                    idx += 1

            # ---- final reduction to a scalar ----
            pr1 = const.tile([pb, 1], fdt)
            nc.vector.tensor_reduce(
                pr1[:], parts[:], axis=mybir.AxisListType.X, op=AL.add)
            acc11 = accp.tile([1, 1], fdt)
            nc.tensor.matmul(acc11[:], ones_col[:], pr1[:],
                             start=True, stop=True)
            tot = const.tile([1, 1], fdt)
            nc.vector.tensor_scalar_mul(tot[:], acc11[:], 0.5)
            nc.sync.dma_start(out_d.ap(), tot[:])

    nc.compile()
    return nc


_NC_CACHE = {}


def _get_nc(gpc=GPC, n=N, pb=PB):
    key = (gpc, n, pb)
    if key not in _NC_CACHE:
        _NC_CACHE[key] = _build_nc(gpc, n, pb)
    return _NC_CACHE[key]


def _expected_pairs(num_graphs, n):
    i = np.repeat(np.arange(n, dtype=np.int64), n)
    j = np.tile(np.arange(n, dtype=np.int64), n)
    keep = i != j
    si, sj = i[keep], j[keep]
    off = (np.arange(num_graphs, dtype=np.int64) * n)[:, None]
    src = (off + si[None, :]).reshape(-1)
    dst = (off + sj[None, :]).reshape(-1)
    return src.astype(np.int32), dst.astype(np.int32)


def _structure_ok(src, dst):
    if src.shape != (NUM_GRAPHS * N * (N - 1),):
        return False
    esrc, edst = _expected_pairs(NUM_GRAPHS, N)
    return np.array_equal(src, esrc) and np.array_equal(dst, edst)


def _fallback_numpy(p, edge_attr, src, dst):
    start = p[src].astype(np.float64)
    end = p[dst].astype(np.float64)
    t12 = ((start - end) ** 2).sum(axis=1)
    l = edge_attr[:, 0].astype(np.float64)
    k = edge_attr[:, 1].astype(np.float64)
    energy = k / 2.0 * (t12 + l * l - 2.0 * l * np.sqrt(t12))
    return np.float32(energy.sum())


def _build_plt_prt(p_core, gpc=GPC, n=N):
    """p_core [gpc*n, 2] f32 -> (plt, prt) [64, 4n] bf16 matmul operands."""
    xb = p_core.reshape(gpc, n, 2).astype(bf16)          # bf16-rounded coords
    xf = xb[..., 0].astype(np.float32)
    yf = xb[..., 1].astype(np.float32)
    r = xf * xf + yf * yf
    rhi = r.astype(bf16)
    r1 = r - rhi.astype(np.float32)
    rmid = r1.astype(bf16)
    r2 = r1 - rmid.astype(np.float32)
    rlo = r2.astype(bf16)
    plt = np.ones((64, 4 * n), dtype=bf16)
    prt = np.ones((64, 4 * n), dtype=bf16)
    feats_l = [xb[..., 0], xb[..., 1], rhi, rmid, rlo]
    feats_r = [(xb[..., 0] * bf16(-2.0)), (xb[..., 1] * bf16(-2.0)),
               None, None, None, rhi, rmid, rlo]
    for g in range(gpc):
        g_, gg = divmod(g, 4)
        cols = slice(gg * n, (gg + 1) * n)
        for f, arr in enumerate(feats_l):
            plt[32 * g_ + f, cols] = arr[g]
        for f, arr in enumerate(feats_r):
            if arr is not None:
                prt[32 * g_ + f, cols] = arr[g]
    return plt, prt


def _build_grids(edge_attr):
    """edge_attr [E,2] f32 -> lk bf16 array [NCORES, GPC, PB, 2, TB, N]."""
    tb = N // PB
    ea = edge_attr.astype(bf16).reshape(NUM_GRAPHS, N * (N - 1), 2)
    offdiag = (~np.eye(N, dtype=bool)).reshape(-1)
    grid = np.zeros((2, NUM_GRAPHS, N * N), dtype=bf16)
    grid[0][:, offdiag] = ea[:, :, 0]
    grid[1][:, offdiag] = ea[:, :, 1]
    # [2, graphs, t, p, j] -> [cores, gpc, p, 2, t*j]
    g5 = grid.reshape(2, NUM_GRAPHS, tb, PB, N)
    lk = np.ascontiguousarray(g5.transpose(1, 3, 0, 2, 4))  # [G, PB, 2, tb, N]
    return lk.reshape(NCORES, GPC, PB, 2, tb * N)


def kernel(p, edge_attr, src, dst):
    p = np.ascontiguousarray(np.asarray(p, dtype=np.float32))
    edge_attr = np.ascontiguousarray(np.asarray(edge_attr, dtype=np.float32))
    src = np.asarray(src, dtype=np.int32)
    dst = np.asarray(dst, dtype=np.int32)

    if not _structure_ok(src, dst):
        return _fallback_numpy(p, edge_attr, src, dst)

    from concourse.bass_utils import run_bass_kernel_spmd

    lk = _build_grids(edge_attr)
    pcs = p.reshape(NCORES, GPC * N, 2)

    nc = _get_nc()
    in_maps = []
    for c in range(NCORES):
        plt, prt = _build_plt_prt(pcs[c])
        in_maps.append({"lk": lk[c], "plin": plt, "prin": prt})
    res = run_bass_kernel_spmd(nc, in_maps, list(range(NCORES)))
    total = sum(float(res.results[c]["out"][0, 0]) for c in range(NCORES))
    return np.float32(total)


if __name__ == "__main__":
    nc = _get_nc()
    print("compiled ok")


# revision 31
# speedup vs baseline: 1.0580x; 1.0335x over previous
"""Trainium2 Bass kernel for the all-pairs spring-energy sum (EnergyLossVectorized).

Contract: kernel(**inputs) takes FULL unsharded inputs (p [32768,2] f32,
edge_attr [E,2] f32, src/dst [E] i32 with E = 64*512*511), returns the FULL
scalar output, distributing across 8 NeuronCores internally.

Strategy: src/dst produced by the reference's setup_inputs() are the
deterministic all-directed-pairs (i != j) indices per graph, in i-major
order.  We verify that structure on the host (falling back to a straight
numpy evaluation if it ever doesn't hold) and then compute the energy with
a gather-free formulation:

  For each graph g (512 nodes), the 512x512 grid D2[i,j] = |p_i - p_j|^2 is
  computed on the tensor engine as a K=8 matmul  D2 = PL^T @ PR with
     PL features: [ x,  y,  rhi, rmid, rlo, 1, 1, 1 ]
     PR features: [-2x, -2y, 1,   1,   1,   rhi, rmid, rlo ]
  where r = x^2 + y^2 of the bf16-rounded coords is carried as three bf16
  limbs, so the PSUM result equals |p_i - p_j|^2 to ~fp32 accuracy (no
  cancellation blowup), guaranteeing D2 >= -1e-5 and sqrt(D2+EPS) NaN-free.

  edge_attr (l, k) is re-laid-out on the host into per-graph [512,512] bf16
  grids with k=0 on the diagonal, interleaved [p, {l,k}, t, j] so each
  graph is a single contiguous 1 MB DMA.  Per half-graph tile [128 x 1024]:
     s  = sqrt(D2 + EPS)            (scalar engine, PSUM -> SBUF bf16)
     u  = s - l                     (DVE / GPSIMD)
     e  = (u ^ 2) * k  + row-sum    (DVE scalar_tensor_tensor, accum_out)
  Per-row partials accumulate in parts[128, 16]; the final reduction is one
  tensor_reduce + a [1x1] ones-matmul + 0.5 scale on device; the host sums
  the 8 per-core scalars.

Memory traffic per core: 8 graphs * 1 MB = 8.4 MB bf16 -> ~24 us roofline.
"""

import os
import sys

import numpy as np

for _p in ("/opt/trn_rl_repo", "/root/.axon_site/_ro/trn_rl_repo"):
    if os.path.isdir(_p) and _p not in sys.path:
        sys.path.insert(0, _p)

import ml_dtypes

bf16 = ml_dtypes.bfloat16

NUM_GRAPHS = 64
N = 512                      # nodes per graph
NCORES = 8
GPC = NUM_GRAPHS // NCORES   # graphs per core = 8
PB = 128                     # partition block (i-tile)
EPS = 1e-5                   # sqrt clamp; D2 >= -1e-5 guaranteed by 3-limb r

# per-half-tile engine assignment (16 half-tiles), tuned from profiles:
# sub u=s-l: D=vector, G=gpsimd ; sq: D/G tensor_mul v=u*k, A=scalar Square
# stt (fused multiply + row-sum): D=vector, G=gpsimd
# invariant: at most one gpsimd op per tile chain (it's 2.3x slower per op)
SUB_PAT = "GGDGGDGGDGGDGGDD"
SQ_PAT = "AAAAAGAAAAAGAAAG"
STT_PAT = "DDDDDDDDDDDDDDDD"


def _build_nc(gpc=GPC, n=N, pb=PB, debug=False):
    """Build + compile the per-core Bass program (SPMD, same on all cores)."""
    import concourse.bass as bass
    import concourse.tile as tile
    from concourse import bacc, mybir

    tb = n // pb             # i-tiles per graph (4)
    th = tb // 2             # halves per graph (2), each [pb, 2, n]
    fdt = mybir.dt.float32
    bdt = mybir.dt.bfloat16
    AF = mybir.ActivationFunctionType
    AL = mybir.AluOpType

    nc = bacc.Bacc("TRN2", target_bir_lowering=False, debug=debug,
                   num_devices=NCORES)

    # lk: [graph, partition, {l,k}, t*j] so one graph = 1 contiguous DMA
    lk_d = nc.dram_tensor("lk", [gpc, pb, 2, tb * n], bdt,
                          kind="ExternalInput")
    pl_d = nc.dram_tensor("plin", [64, 4 * n], bdt, kind="ExternalInput")
    pr_d = nc.dram_tensor("prin", [64, 4 * n], bdt, kind="ExternalInput")
    out_d = nc.dram_tensor("out", [1, 1], fdt, kind="ExternalOutput")

    lk = lk_d.ap()

    with tile.TileContext(nc) as tc:
        with (
            tc.tile_pool(name="const", bufs=1) as const,
            tc.tile_pool(name="lkp", bufs=4) as lkp,
            tc.tile_pool(name="work", bufs=4) as work,
            tc.tile_pool(name="psum", bufs=3, space="PSUM") as psum,
            tc.tile_pool(name="accp", bufs=1, space="PSUM") as accp,
        ):
            # host-precomputed matmul operands (see _build_plt_prt)
            plt = const.tile([64, 4 * n], bdt)
            prt = const.tile([64, 4 * n], bdt)
            nc.sync.dma_start(plt[:], pl_d.ap())
            nc.sync.dma_start(prt[:], pr_d.ap())

            ones_col = const.tile([pb, 1], fdt)
            nc.vector.memset(ones_col[:], 1.0)
            eps_col = const.tile([pb, 1], fdt)
            nc.vector.memset(eps_col[:], EPS)
            zero_col = const.tile([pb, 1], fdt)
            nc.vector.memset(zero_col[:], 0.0)

            parts = const.tile([pb, gpc * th], fdt)

            nh = 2 * n           # half-graph free width (flat 2D, step-1)
            idx = 0
            for g in range(gpc):
                g_, gg = divmod(g, 4)
                lkt = lkp.tile([pb, 2, tb * n], bdt)
                nc.sync.dma_start(lkt[:], lk[g])
                for h in range(th):
                    ps = psum.tile([pb, nh], fdt)
                    for tt in range(2):
                        t = 2 * h + tt
                        nc.tensor.matmul(
                            ps[:, tt * n:(tt + 1) * n],
                            plt[32 * g_:32 * g_ + 8,
                                gg * n + t * pb: gg * n + (t + 1) * pb],
                            prt[32 * g_:32 * g_ + 8, gg * n:(gg + 1) * n],
                            start=True, stop=True,
                        )
                    s = work.tile([pb, nh], bdt, tag="s")
                    nc.scalar.activation(s[:], ps[:], AF.Sqrt, bias=eps_col[:])
                    u = work.tile([pb, nh], bdt, tag="u")
                    lsl = lkt[:, 0, h * nh:(h + 1) * nh]
                    ksl = lkt[:, 1, h * nh:(h + 1) * nh]
                    if SUB_PAT[idx] == "G":
                        nc.gpsimd.tensor_sub(u[:], s[:], lsl)
                    else:
                        nc.vector.tensor_sub(u[:], s[:], lsl)
                    # e = k*u^2 with fused row-sum into parts[:, idx]
                    v = work.tile([pb, nh], bdt, tag="v")
                    e = work.tile([pb, nh], bdt, tag="e")
                    sq = SQ_PAT[idx]
                    if sq == "A":
                        nc.scalar.activation(v[:], u[:], AF.Square,
                                             bias=zero_col[:])
                        a0, a1 = v[:], ksl
                    else:
                        if sq == "G":
                            nc.gpsimd.tensor_mul(v[:], u[:], ksl)
                        else:
                            nc.vector.tensor_mul(v[:], u[:], ksl)
                        a0, a1 = v[:], u[:]
                    stt_eng = (nc.gpsimd if STT_PAT[idx] == "G"
                               else nc.vector)
                    stt_eng.scalar_tensor_tensor(
                        e[:], a0, 1.0, a1,
                        op0=AL.mult, op1=AL.mult,
                        accum_out=parts[:, idx:idx + 1],
                    )
                    idx += 1

            # ---- final reduction to a scalar ----
            pr1 = const.tile([pb, 1], fdt)
            nc.vector.tensor_reduce(
                pr1[:], parts[:], axis=mybir.AxisListType.X, op=AL.add)
            acc11 = accp.tile([1, 1], fdt)
            nc.tensor.matmul(acc11[:], ones_col[:], pr1[:],
                             start=True, stop=True)
            tot = const.tile([1, 1], fdt)
            nc.vector.tensor_scalar_mul(tot[:], acc11[:], 0.5)
            nc.sync.dma_start(out_d.ap(), tot[:])

    nc.compile()
    return nc


_NC_CACHE = {}


def _get_nc(gpc=GPC, n=N, pb=PB):
    key = (gpc, n, pb)
    if key not in _NC_CACHE:
        _NC_CACHE[key] = _build_nc(gpc, n, pb)
    return _NC_CACHE[key]


def _expected_pairs(num_graphs, n):
    i = np.repeat(np.arange(n, dtype=np.int64), n)
    j = np.tile(np.arange(n, dtype=np.int64), n)
    keep = i != j
    si, sj = i[keep], j[keep]
    off = (np.arange(num_graphs, dtype=np.int64) * n)[:, None]
    src = (off + si[None, :]).reshape(-1)
    dst = (off + sj[None, :]).reshape(-1)
    return src.astype(np.int32), dst.astype(np.int32)


def _structure_ok(src, dst):
    if src.shape != (NUM_GRAPHS * N * (N - 1),):
        return False
    esrc, edst = _expected_pairs(NUM_GRAPHS, N)
    return np.array_equal(src, esrc) and np.array_equal(dst, edst)


def _fallback_numpy(p, edge_attr, src, dst):
    start = p[src].astype(np.float64)
    end = p[dst].astype(np.float64)
    t12 = ((start - end) ** 2).sum(axis=1)
    l = edge_attr[:, 0].astype(np.float64)
    k = edge_attr[:, 1].astype(np.float64)
    energy = k / 2.0 * (t12 + l * l - 2.0 * l * np.sqrt(t12))
    return np.float32(energy.sum())


def _build_plt_prt(p_core, gpc=GPC, n=N):
    """p_core [gpc*n, 2] f32 -> (plt, prt) [64, 4n] bf16 matmul operands."""
    xb = p_core.reshape(gpc, n, 2).astype(bf16)          # bf16-rounded coords
    xf = xb[..., 0].astype(np.float32)
    yf = xb[..., 1].astype(np.float32)
    r = xf * xf + yf * yf
    rhi = r.astype(bf16)
    r1 = r - rhi.astype(np.float32)
    rmid = r1.astype(bf16)
    r2 = r1 - rmid.astype(np.float32)
    rlo = r2.astype(bf16)
    plt = np.ones((64, 4 * n), dtype=bf16)
    prt = np.ones((64, 4 * n), dtype=bf16)
    feats_l = [xb[..., 0], xb[..., 1], rhi, rmid, rlo]
    feats_r = [(xb[..., 0] * bf16(-2.0)), (xb[..., 1] * bf16(-2.0)),
               None, None, None, rhi, rmid, rlo]
    for g in range(gpc):
        g_, gg = divmod(g, 4)
        cols = slice(gg * n, (gg + 1) * n)
        for f, arr in enumerate(feats_l):
            plt[32 * g_ + f, cols] = arr[g]
        for f, arr in enumerate(feats_r):
            if arr is not None:
                prt[32 * g_ + f, cols] = arr[g]
    return plt, prt


def _build_grids(edge_attr):
    """edge_attr [E,2] f32 -> lk bf16 array [NCORES, GPC, PB, 2, TB, N]."""
    tb = N // PB
    ea = edge_attr.astype(bf16).reshape(NUM_GRAPHS, N * (N - 1), 2)
    offdiag = (~np.eye(N, dtype=bool)).reshape(-1)
    grid = np.zeros((2, NUM_GRAPHS, N * N), dtype=bf16)
    grid[0][:, offdiag] = ea[:, :, 0]
    grid[1][:, offdiag] = ea[:, :, 1]
    # [2, graphs, t, p, j] -> [cores, gpc, p, 2, t*j]
    g5 = grid.reshape(2, NUM_GRAPHS, tb, PB, N)
    lk = np.ascontiguousarray(g5.transpose(1, 3, 0, 2, 4))  # [G, PB, 2, tb, N]
    return lk.reshape(NCORES, GPC, PB, 2, tb * N)


def kernel(p, edge_attr, src, dst):
    p = np.ascontiguousarray(np.asarray(p, dtype=np.float32))
    edge_attr = np.ascontiguousarray(np.asarray(edge_attr, dtype=np.float32))
    src = np.asarray(src, dtype=np.int32)
    dst = np.asarray(dst, dtype=np.int32)

    if not _structure_ok(src, dst):
        return _fallback_numpy(p, edge_attr, src, dst)

    from concourse.bass_utils import run_bass_kernel_spmd

    lk = _build_grids(edge_attr)
    pcs = p.reshape(NCORES, GPC * N, 2)

    nc = _get_nc()
    in_maps = []
    for c in range(NCORES):
        plt, prt = _build_plt_prt(pcs[c])
        in_maps.append({"lk": lk[c], "plin": plt, "prin": prt})
    res = run_bass_kernel_spmd(nc, in_maps, list(range(NCORES)))
    total = sum(float(res.results[c]["out"][0, 0]) for c in range(NCORES))
    return np.float32(total)


if __name__ == "__main__":
    nc = _get_nc()
    print("compiled ok")
